# revision 11
# baseline (speedup 1.0000x reference)
"""Ewald reciprocal-space kernel for Trainium2 (8 NeuronCores, SPMD) — v3.

Math (per batch b):
    s        = cell_inv @ x          (fractional coords)
    theta    = 2*pi * (kvec . s)     (B, N, NK) phases
    S_re/S_im= sum_n q_n {cos,sin}(theta)          (structure factor)
    recip_n  = sum_k expfac_k (S_re cos + S_im sin)
    phi      = recip * BOHR/(pi*V) - q * 2*bewald*BOHR/sqrt(pi)
    returns (0.5*q*phi, phi)

Sharding: 8 cores = 2 batches x 4 k-shards (1024 k-vectors each). Each core
computes its full-N, shard-K contribution to recip with no collectives; host
sums the 4 shard partials per batch and applies the final affine.

Device pipeline per core (N=4096 as 32 chunks of 128 partitions):
  u  = x . kmod (kmod = Cinv^T k)       fp32r matmul into PSUM
  rn = (u + M) - M                      magic round; DVE or ACT-assisted
  f  = rn - u  in [-1/2, 1/2]           DVE scalar_tensor_tensor, fp16 out
  g  = |f|     (uint16 AND 0x7FFF)      DVE 4x-mode fp16
  h  = g - 1/4                          GPSIMD (pool), fp16
  sin(theta) = Sin(-2pi f), cos(theta) = Sin(-2pi h)   one ACT Sin per 2 chunks
  S rows: 4 PE matmuls/chunk, out [1, 512] parked at quadrant partition 32q
     (4 concurrent PSUM groups share one 2KB region on disjoint partitions)
  cs chunks DMA-transposed (xbar) into csT[k-slice partitions, n free]
  S extraction: 4 single-partition row copies (DVE+ACT), then 16 tiny PE
     transposes [1,128]->[128,1] into one PSUM group; w = efT * S^T (DVE)
  recip: free-size-1 weight-stationary PE matmuls (lhsT = csT block,
     rhs = w column), accumulated over 16 k-slices; PSUM ring reuse with
     per-pair copy-out, recip emitted as [128, 32]
"""

import math
from contextlib import ExitStack

import numpy as np

BOHR = 1.8897261258369282

B, N, NK = 2, 4096, 4096
NCORES = 8
KSH = NK // 4          # k-vectors per core
NCH = N // 128         # 32 n-chunks
CW = 2 * KSH           # cs chunk width: [sin | cos]
NSL = CW // 128        # 16 k-slices per chunk
MAGIC = 12582912.0     # 1.5 * 2**23: fp32 round-to-nearest-integer
NEG2PI = -6.28318452835083  # two fp32 ulps below 2*pi

_PROG = {}


def _build_program(reps: int = 1, stage: str = 'full', n_assist: int = 14,
                   mm_bufs: int = 3, rn_bufs: int = 2, cs_bufs: int = 2,
                   extr_act: int = 4, p2_act: int = 1, assist_last: int = NCH,
                   assist0: bool = False,
                   split_sins: tuple = (0, 1, NCH // 2 - 2, NCH // 2 - 1),
                   out_pieces: int = 1, p2_desc: bool = False,
                   assist_set: frozenset = None, pre_u: int = 3,
                   hoist: bool = False, extr_rev: bool = False):
    import concourse.bass as bass
    import concourse.bacc as bacc
    import concourse.tile as tile
    import concourse.mybir as mybir

    F32 = mybir.dt.float32
    F32R = mybir.dt.float32r
    F16 = mybir.dt.float16
    U16 = mybir.dt.uint16
    ADD = mybir.AluOpType.add
    SUB = mybir.AluOpType.subtract
    MULT = mybir.AluOpType.mult
    AND = mybir.AluOpType.bitwise_and
    SIN = mybir.ActivationFunctionType.Sin
    COPY = mybir.ActivationFunctionType.Copy

    nc = bacc.Bacc(trn_type="TRN2", target_bir_lowering=False, debug=False)

    coordsT_d = nc.dram_tensor("coordsT", [3, N], F32, kind="ExternalInput").ap()
    qT_d = nc.dram_tensor("qT", [128, NCH], F32, kind="ExternalInput").ap()
    kmodT_d = nc.dram_tensor("kmodT", [3, KSH], F32, kind="ExternalInput").ap()
    efT_d = nc.dram_tensor("efT", [128, NSL], F32, kind="ExternalInput").ap()
    recip_d = nc.dram_tensor("recip", [128, NCH], F32, kind="ExternalOutput").ap()

    # chunks whose rounding runs on the scalar engine (DVE/ACT balance);
    # assist_last bounds the last assisted chunk so the ACT tail is Sin-only
    ASSIST = {int((i + 0.5) * assist_last / n_assist) for i in range(n_assist)}
    if assist0:
        ASSIST.add(0)
    if assist_set is not None:
        ASSIST = set(assist_set)

    with tile.TileContext(nc) as tc, ExitStack() as ctx:
        const = ctx.enter_context(tc.tile_pool(name="const", bufs=1))
        pu = ctx.enter_context(tc.tile_pool(name="pu", bufs=3, space="PSUM"))
        pacc = ctx.enter_context(tc.tile_pool(name="pacc", bufs=1, space="PSUM"))
        wk_rn = ctx.enter_context(tc.tile_pool(name="wk_rn", bufs=rn_bufs))
        wk_mm = ctx.enter_context(tc.tile_pool(name="wk_mm", bufs=mm_bufs))
        wk_cs = ctx.enter_context(tc.tile_pool(name="wk_cs", bufs=cs_bufs))
        wk_out = ctx.enter_context(tc.tile_pool(name="wk_out", bufs=1))

        # ---- load inputs (first chunk's dependencies in tiny pieces first) ----
        cts = const.tile([3, N], F32R)
        kmod = const.tile([3, KSH], F32R, name="kmod")
        nc.sync.dma_start(out=kmod[:, 0:512], in_=kmodT_d[:, 0:512].bitcast(F32R))
        nc.scalar.dma_start(out=cts[:, 0:384], in_=coordsT_d[:, 0:384].bitcast(F32R))
        nc.sync.dma_start(out=kmod[:, 512:1024],
                          in_=kmodT_d[:, 512:1024].bitcast(F32R))
        nc.sync.dma_start(out=cts[:, 384:1024],
                          in_=coordsT_d[:, 384:1024].bitcast(F32R))
        for hh in range(1024, N, 1024):
            nc.sync.dma_start(
                out=cts[:, hh:hh + 1024],
                in_=coordsT_d[:, hh:hh + 1024].bitcast(F32R),
            )
        qt = const.tile([128, NCH], F32)
        nc.sync.dma_start(out=qt[:, :], in_=qT_d)
        efT = const.tile([128, NSL], F32)
        nc.sync.dma_start(out=efT[:, :], in_=efT_d)
        qt16 = const.tile([128, NCH], F16)
        one1 = const.tile([1, 1], F32)

        # persistent: the transposed trig store and the S quadrant rows
        csT = const.tile([128, NSL, N], F16)   # [k-in-slice][slice j][n]
        sgrid = pacc.tile([128, 512], F32, tag="sg")

        vs = {}

        def emit_u(t):
            u_ps = pu.tile([128, KSH], F32, tag="u", name=f"u{t}")
            for hh in range(0, KSH, 512):
                nc.tensor.matmul(
                    u_ps[:, hh:hh + 512],
                    lhsT=cts[:, 128 * t:128 * (t + 1)],
                    rhs=kmod[:, hh:hh + 512],
                    start=True, stop=True,
                )
            if hoist and t in ASSIST and stage != 'rr0':
                # emit the ACT rounding assist at the producer so it can fill
                # ACT queue gaps ahead of the pair's Sin
                rnh = wk_rn.tile([128, KSH], F32, tag="rn", name=f"rnh{t}")
                nc.scalar.activation(rnh[:, :], u_ps[:, :], COPY,
                                     bias=MAGIC, scale=1.0)
                vs[t] = rnh
            return u_ps

        us = {t: emit_u(t) for t in range(pre_u)}
        for s in range(NCH // 2):
            mmp = wk_mm.tile([128, 2 * CW], F16, tag="mm", name=f"mm{s}")
            for c in (0, 1):
                t = 2 * s + c
                u_ps = us.pop(t)
                fsl = mmp[:, CW * c:CW * c + KSH]            # f (sin half)
                hsl = mmp[:, CW * c + KSH:CW * (c + 1)]      # h (cos half)
                if t in vs:
                    rn = vs.pop(t)
                    s0 = MAGIC
                elif t in ASSIST and stage != 'rr0':
                    # ACT: v = u + M (rounds); DVE stt: (v - M) - u
                    rn = wk_rn.tile([128, KSH], F32, tag="rn", name=f"rn{t}")
                    nc.scalar.activation(rn[:, :], u_ps[:, :], COPY,
                                         bias=MAGIC, scale=1.0)
                    s0 = MAGIC
                else:
                    rn = None
                    rn = wk_rn.tile([128, KSH], F32, tag="rn", name=f"rn{t}")
                    nc.vector.tensor_scalar(
                        out=rn[:, :], in0=u_ps[:, :],
                        scalar1=MAGIC, scalar2=MAGIC, op0=ADD, op1=SUB,
                    )
                    s0 = 0.0
                nc.vector.scalar_tensor_tensor(
                    out=fsl, in0=rn[:, :], scalar=s0, in1=u_ps[:, :],
                    op0=ADD if s0 == 0.0 else SUB, op1=SUB,
                )
                # g = |f| via uint16 AND; h = g - 1/4 on the pool engine
                nc.vector.tensor_scalar(
                    out=hsl.bitcast(U16), in0=fsl.bitcast(U16),
                    scalar1=0x7FFF, scalar2=None, op0=AND,
                )
                nc.gpsimd.tensor_scalar(
                    out=hsl, in0=hsl, scalar1=0.25, scalar2=None, op0=SUB,
                )
                tn = t + pre_u
                if tn < NCH:
                    us[tn] = emit_u(tn)
            if s == 0:
                nc.vector.tensor_copy(qt16[:, :], qt[:, :])
                nc.vector.memset(one1[:, :], 1.0)
            if stage == 'rr':
                continue
            # sin(theta) = Sin(-2pi f); cos(theta) = Sin(-2pi h)
            csp = wk_cs.tile([128, 2 * CW], F16, tag="cs", name=f"cs{s}")
            if s in split_sins:
                nc.scalar.activation(csp[:, 0:CW], mmp[:, 0:CW], SIN,
                                     bias=0.0, scale=NEG2PI)
            else:
                nc.scalar.activation(csp[:, :], mmp[:, :], SIN,
                                     bias=0.0, scale=NEG2PI)
            for c in (0, 1):
                t = 2 * s + c
                if s in split_sins and c == 1:
                    nc.scalar.activation(csp[:, CW:], mmp[:, CW:], SIN,
                                         bias=0.0, scale=NEG2PI)
                for qd in range(4):
                    nc.tensor.matmul(
                        sgrid[32 * qd:32 * qd + 1, 0:512],
                        lhsT=qt16[:, t:t + 1],
                        rhs=csp[:, CW * c + 512 * qd:CW * c + 512 * (qd + 1)],
                        start=(t == 0), stop=(t == NCH - 1),
                        tile_position=(0, 32 * qd),
                    )
                if stage != 'act':
                    # csT[p, j, 128t + n] = csp[n, CW*c + 128j + p]
                    nc.sync.dma_start_transpose(
                        out=csT[:, :, 128 * t:128 * (t + 1)],
                        in_=csp[:, CW * c:CW * (c + 1)],
                    )

        if stage != 'full':
            zz = wk_out.tile([128, NCH], F32, name="zz")
            nc.vector.memset(zz[:, :], 0.0)
            nc.sync.dma_start(out=recip_d, in_=zz[:, :])
        else:
            # ---- S extraction: 4 quadrant rows -> SBUF (DVE + ACT) ----
            sgq0 = const.tile([1, 512], F32, name="sgq0")
            sgq1 = const.tile([1, 512], F32, name="sgq1")
            sgq2 = const.tile([1, 512], F32, name="sgq2")
            sgq3 = const.tile([1, 512], F32, name="sgq3")
            sgq = [sgq0, sgq1, sgq2, sgq3]
            wtp = pacc.tile([128, 512], F32, tag="wtp")
            for qd in range(4):
                on_act = (qd < extr_act) != extr_rev
                if not on_act:
                    nc.vector.tensor_copy(sgq[qd][:, :],
                                          sgrid[32 * qd:32 * qd + 1, 0:512])
                else:
                    nc.scalar.copy(sgq[qd][:, :],
                                   sgrid[32 * qd:32 * qd + 1, 0:512])
                for aa in range(4):
                    j = 4 * qd + aa
                    nc.tensor.matmul(
                        wtp[:, j:j + 1],
                        lhsT=sgq[qd][0:1, 128 * aa:128 * (aa + 1)],
                        rhs=one1[:, :],
                        is_transpose=True, start=(j == 0), stop=(j == NSL - 1),
                    )
            wcolT = const.tile([128, NSL], F16, tag="wcolT")
            nc.vector.tensor_tensor(
                out=wcolT[:, :], in0=wtp[:, 0:NSL], in1=efT[:, :], op=MULT,
            )

            # ---- pass 2: recip[n] = sum_j csT[:, j, n]^T wcolT[:, j] ----
            rr = wk_out.tile([128, NCH], F32, name="rr")
            p2_order = list(range(NCH // 2))
            if p2_desc:
                p2_order = p2_order[::-1]
            for m in p2_order:
                rp = pu.tile([128, KSH], F32, tag="u", name=f"rp{m}")
                for c in (0, 1):
                    t = 2 * m + c
                    for j in range(NSL):
                        nc.tensor.matmul(
                            rp[:, 512 * c:512 * c + 1],
                            lhsT=csT[:, j, 128 * t:128 * (t + 1)],
                            rhs=wcolT[:, j:j + 1],
                            start=(j == 0), stop=(j == NSL - 1),
                        )
                if m % 4 < p2_act:
                    nc.scalar.copy(rr[:, 2 * m:2 * m + 2], rp[:, 0:KSH:512])
                else:
                    nc.vector.tensor_copy(rr[:, 2 * m:2 * m + 2],
                                          rp[:, 0:KSH:512])
                if out_pieces == 2 and m == 11:
                    nc.sync.dma_start(out=recip_d[:, 0:24], in_=rr[:, 0:24])
            if out_pieces == 2:
                nc.sync.dma_start(out=recip_d[:, 24:], in_=rr[:, 24:])
            else:
                nc.sync.dma_start(out=recip_d, in_=rr[:, :])

    nc.compile()
    return nc


def _get_prog(reps: int = 1, stage: str = "full", **kw):
    key = (reps, stage, tuple(sorted(kw.items())))
    if key not in _PROG:
        _PROG[key] = _build_program(reps, stage, **kw)
    return _PROG[key]


def _make_in_maps(coords, q, cell_inv, kvec, expfac):
    in_maps = []
    for c in range(NCORES):
        b, ks = divmod(c, NCORES // B)
        sl = slice(KSH * ks, KSH * (ks + 1))
        # efT[p, j] = expfac_shard[(128j + p) mod 1024]
        ef = np.asarray(expfac[sl], dtype=np.float32).reshape(8, 128).T  # [128, 8]
        efT = np.concatenate([ef, ef], axis=1)                           # [128, 16]
        in_maps.append({
            "coordsT": np.ascontiguousarray(coords[b].T, dtype=np.float32),
            "qT": np.ascontiguousarray(q[b].reshape(NCH, 128).T, dtype=np.float32),
            "kmodT": np.ascontiguousarray(
                cell_inv.astype(np.float32).T @ kvec[sl].T.astype(np.float32)),
            "efT": np.ascontiguousarray(efT),
        })
    return in_maps


def _finalize(results, q, volume, bewald):
    recip = np.zeros((B, N), np.float32)
    for c in range(NCORES):
        b = c // (NCORES // B)
        recip[b] += results[c]["recip"].T.reshape(-1)
    scale1 = np.float32(BOHR / (math.pi * float(volume[0])))
    scale2 = np.float32(2.0 * float(bewald[0]) * BOHR / math.sqrt(math.pi))
    phi = (recip * scale1 - q.astype(np.float32) * scale2).astype(np.float32)
    e = (np.float32(0.5) * q.astype(np.float32) * phi).astype(np.float32)
    return e, phi


def kernel(coords, q, cell_inv, kvec, expfac, volume, bewald):
    from concourse.bass_utils import run_bass_kernel_spmd

    nc = _get_prog()
    in_maps = _make_in_maps(coords, q, cell_inv, kvec, expfac)
    res = run_bass_kernel_spmd(nc, in_maps, list(range(NCORES))).results
    return _finalize(res, q, volume, bewald)


# revision 12
# speedup vs baseline: 1.0028x; 1.0028x over previous
"""Ewald reciprocal-space kernel for Trainium2 (8 NeuronCores, SPMD) — v3.

Math (per batch b):
    s        = cell_inv @ x          (fractional coords)
    theta    = 2*pi * (kvec . s)     (B, N, NK) phases
    S_re/S_im= sum_n q_n {cos,sin}(theta)          (structure factor)
    recip_n  = sum_k expfac_k (S_re cos + S_im sin)
    phi      = recip * BOHR/(pi*V) - q * 2*bewald*BOHR/sqrt(pi)
    returns (0.5*q*phi, phi)

Sharding: 8 cores = 2 batches x 4 k-shards (1024 k-vectors each). Each core
computes its full-N, shard-K contribution to recip with no collectives; host
sums the 4 shard partials per batch and applies the final affine.

Device pipeline per core (N=4096 as 32 chunks of 128 partitions):
  u  = x . kmod (kmod = Cinv^T k)       fp32r matmul into PSUM
  rn = (u + M) - M                      magic round; DVE or ACT-assisted
  f  = rn - u  in [-1/2, 1/2]           DVE scalar_tensor_tensor, fp16 out
  g  = |f|     (uint16 AND 0x7FFF)      DVE 4x-mode fp16
  h  = g - 1/4                          GPSIMD (pool), fp16
  sin(theta) = Sin(-2pi f), cos(theta) = Sin(-2pi h)   one ACT Sin per 2 chunks
  S rows: 4 PE matmuls/chunk, out [1, 512] parked at quadrant partition 32q
     (4 concurrent PSUM groups share one 2KB region on disjoint partitions)
  cs chunks DMA-transposed (xbar) into csT[k-slice partitions, n free]
  S extraction: 4 single-partition row copies (DVE+ACT), then 16 tiny PE
     transposes [1,128]->[128,1] into one PSUM group; w = efT * S^T (DVE)
  recip: free-size-1 weight-stationary PE matmuls (lhsT = csT block,
     rhs = w column), accumulated over 16 k-slices; PSUM ring reuse with
     per-pair copy-out, recip emitted as [128, 32]
"""

import math
from contextlib import ExitStack

import numpy as np

BOHR = 1.8897261258369282

B, N, NK = 2, 4096, 4096
NCORES = 8
KSH = NK // 4          # k-vectors per core
NCH = N // 128         # 32 n-chunks
CW = 2 * KSH           # cs chunk width: [sin | cos]
NSL = CW // 128        # 16 k-slices per chunk
MAGIC = 12582912.0     # 1.5 * 2**23: fp32 round-to-nearest-integer
NEG2PI = -6.28318452835083  # two fp32 ulps below 2*pi

_PROG = {}


def _build_program(reps: int = 1, stage: str = 'full', n_assist: int = 14,
                   mm_bufs: int = 3, rn_bufs: int = 2, cs_bufs: int = 2,
                   extr_act: int = 4, p2_act: int = 1, assist_last: int = NCH,
                   assist0: bool = False,
                   split_sins: tuple = (NCH // 2 - 2, NCH // 2 - 1),
                   out_pieces: int = 1, p2_desc: bool = False,
                   assist_set: frozenset = None, pre_u: int = 3,
                   hoist: bool = False, extr_rev: bool = False):
    import concourse.bass as bass
    import concourse.bacc as bacc
    import concourse.tile as tile
    import concourse.mybir as mybir

    F32 = mybir.dt.float32
    F32R = mybir.dt.float32r
    F16 = mybir.dt.float16
    U16 = mybir.dt.uint16
    ADD = mybir.AluOpType.add
    SUB = mybir.AluOpType.subtract
    MULT = mybir.AluOpType.mult
    AND = mybir.AluOpType.bitwise_and
    SIN = mybir.ActivationFunctionType.Sin
    COPY = mybir.ActivationFunctionType.Copy

    nc = bacc.Bacc(trn_type="TRN2", target_bir_lowering=False, debug=False)

    coordsT_d = nc.dram_tensor("coordsT", [3, N], F32, kind="ExternalInput").ap()
    qT_d = nc.dram_tensor("qT", [128, NCH], F32, kind="ExternalInput").ap()
    kmodT_d = nc.dram_tensor("kmodT", [3, KSH], F32, kind="ExternalInput").ap()
    efT_d = nc.dram_tensor("efT", [128, NSL], F32, kind="ExternalInput").ap()
    recip_d = nc.dram_tensor("recip", [128, NCH], F32, kind="ExternalOutput").ap()

    # chunks whose rounding runs on the scalar engine (DVE/ACT balance);
    # assist_last bounds the last assisted chunk so the ACT tail is Sin-only
    ASSIST = {int((i + 0.5) * assist_last / n_assist) for i in range(n_assist)}
    if assist0:
        ASSIST.add(0)
    if assist_set is not None:
        ASSIST = set(assist_set)

    with tile.TileContext(nc) as tc, ExitStack() as ctx:
        const = ctx.enter_context(tc.tile_pool(name="const", bufs=1))
        pu = ctx.enter_context(tc.tile_pool(name="pu", bufs=3, space="PSUM"))
        pacc = ctx.enter_context(tc.tile_pool(name="pacc", bufs=1, space="PSUM"))
        wk_rn = ctx.enter_context(tc.tile_pool(name="wk_rn", bufs=rn_bufs))
        wk_mm = ctx.enter_context(tc.tile_pool(name="wk_mm", bufs=mm_bufs))
        wk_cs = ctx.enter_context(tc.tile_pool(name="wk_cs", bufs=cs_bufs))
        wk_out = ctx.enter_context(tc.tile_pool(name="wk_out", bufs=1))

        # ---- load inputs (first chunk's dependencies in tiny pieces first) ----
        cts = const.tile([3, N], F32R)
        kmod = const.tile([3, KSH], F32R, name="kmod")
        nc.sync.dma_start(out=kmod[:, 0:512], in_=kmodT_d[:, 0:512].bitcast(F32R))
        nc.scalar.dma_start(out=cts[:, 0:384], in_=coordsT_d[:, 0:384].bitcast(F32R))
        nc.sync.dma_start(out=kmod[:, 512:1024],
                          in_=kmodT_d[:, 512:1024].bitcast(F32R))
        nc.sync.dma_start(out=cts[:, 384:1024],
                          in_=coordsT_d[:, 384:1024].bitcast(F32R))
        for hh in range(1024, N, 1024):
            nc.sync.dma_start(
                out=cts[:, hh:hh + 1024],
                in_=coordsT_d[:, hh:hh + 1024].bitcast(F32R),
            )
        qt = const.tile([128, NCH], F32)
        nc.sync.dma_start(out=qt[:, :], in_=qT_d)
        efT = const.tile([128, NSL], F32)
        nc.sync.dma_start(out=efT[:, :], in_=efT_d)
        qt16 = const.tile([128, NCH], F16)
        one1 = const.tile([1, 1], F32)

        # persistent: the transposed trig store and the S quadrant rows
        csT = const.tile([128, NSL, N], F16)   # [k-in-slice][slice j][n]
        sgrid = pacc.tile([128, 512], F32, tag="sg")

        vs = {}

        def emit_u(t):
            u_ps = pu.tile([128, KSH], F32, tag="u", name=f"u{t}")
            for hh in range(0, KSH, 512):
                nc.tensor.matmul(
                    u_ps[:, hh:hh + 512],
                    lhsT=cts[:, 128 * t:128 * (t + 1)],
                    rhs=kmod[:, hh:hh + 512],
                    start=True, stop=True,
                )
            if hoist and t in ASSIST and stage != 'rr0':
                # emit the ACT rounding assist at the producer so it can fill
                # ACT queue gaps ahead of the pair's Sin
                rnh = wk_rn.tile([128, KSH], F32, tag="rn", name=f"rnh{t}")
                nc.scalar.activation(rnh[:, :], u_ps[:, :], COPY,
                                     bias=MAGIC, scale=1.0)
                vs[t] = rnh
            return u_ps

        us = {t: emit_u(t) for t in range(pre_u)}
        for s in range(NCH // 2):
            mmp = wk_mm.tile([128, 2 * CW], F16, tag="mm", name=f"mm{s}")
            for c in (0, 1):
                t = 2 * s + c
                u_ps = us.pop(t)
                fsl = mmp[:, CW * c:CW * c + KSH]            # f (sin half)
                hsl = mmp[:, CW * c + KSH:CW * (c + 1)]      # h (cos half)
                if t in vs:
                    rn = vs.pop(t)
                    s0 = MAGIC
                elif t in ASSIST and stage != 'rr0':
                    # ACT: v = u + M (rounds); DVE stt: (v - M) - u
                    rn = wk_rn.tile([128, KSH], F32, tag="rn", name=f"rn{t}")
                    nc.scalar.activation(rn[:, :], u_ps[:, :], COPY,
                                         bias=MAGIC, scale=1.0)
                    s0 = MAGIC
                else:
                    rn = None
                    rn = wk_rn.tile([128, KSH], F32, tag="rn", name=f"rn{t}")
                    nc.vector.tensor_scalar(
                        out=rn[:, :], in0=u_ps[:, :],
                        scalar1=MAGIC, scalar2=MAGIC, op0=ADD, op1=SUB,
                    )
                    s0 = 0.0
                nc.vector.scalar_tensor_tensor(
                    out=fsl, in0=rn[:, :], scalar=s0, in1=u_ps[:, :],
                    op0=ADD if s0 == 0.0 else SUB, op1=SUB,
                )
                # g = |f| via uint16 AND; h = g - 1/4 on the pool engine
                nc.vector.tensor_scalar(
                    out=hsl.bitcast(U16), in0=fsl.bitcast(U16),
                    scalar1=0x7FFF, scalar2=None, op0=AND,
                )
                nc.gpsimd.tensor_scalar(
                    out=hsl, in0=hsl, scalar1=0.25, scalar2=None, op0=SUB,
                )
                tn = t + pre_u
                if tn < NCH:
                    us[tn] = emit_u(tn)
            if s == 0:
                nc.vector.tensor_copy(qt16[:, :], qt[:, :])
                nc.vector.memset(one1[:, :], 1.0)
            if stage == 'rr':
                continue
            # sin(theta) = Sin(-2pi f); cos(theta) = Sin(-2pi h)
            csp = wk_cs.tile([128, 2 * CW], F16, tag="cs", name=f"cs{s}")
            if s in split_sins:
                nc.scalar.activation(csp[:, 0:CW], mmp[:, 0:CW], SIN,
                                     bias=0.0, scale=NEG2PI)
            else:
                nc.scalar.activation(csp[:, :], mmp[:, :], SIN,
                                     bias=0.0, scale=NEG2PI)
            for c in (0, 1):
                t = 2 * s + c
                if s in split_sins and c == 1:
                    nc.scalar.activation(csp[:, CW:], mmp[:, CW:], SIN,
                                         bias=0.0, scale=NEG2PI)
                for qd in range(4):
                    nc.tensor.matmul(
                        sgrid[32 * qd:32 * qd + 1, 0:512],
                        lhsT=qt16[:, t:t + 1],
                        rhs=csp[:, CW * c + 512 * qd:CW * c + 512 * (qd + 1)],
                        start=(t == 0), stop=(t == NCH - 1),
                        tile_position=(0, 32 * qd),
                    )
                if stage != 'act':
                    # csT[p, j, 128t + n] = csp[n, CW*c + 128j + p]
                    nc.sync.dma_start_transpose(
                        out=csT[:, :, 128 * t:128 * (t + 1)],
                        in_=csp[:, CW * c:CW * (c + 1)],
                    )

        if stage != 'full':
            zz = wk_out.tile([128, NCH], F32, name="zz")
            nc.vector.memset(zz[:, :], 0.0)
            nc.sync.dma_start(out=recip_d, in_=zz[:, :])
        else:
            # ---- S extraction: 4 quadrant rows -> SBUF (DVE + ACT) ----
            sgq0 = const.tile([1, 512], F32, name="sgq0")
            sgq1 = const.tile([1, 512], F32, name="sgq1")
            sgq2 = const.tile([1, 512], F32, name="sgq2")
            sgq3 = const.tile([1, 512], F32, name="sgq3")
            sgq = [sgq0, sgq1, sgq2, sgq3]
            wtp = pacc.tile([128, 512], F32, tag="wtp")
            for qd in range(4):
                on_act = (qd < extr_act) != extr_rev
                if not on_act:
                    nc.vector.tensor_copy(sgq[qd][:, :],
                                          sgrid[32 * qd:32 * qd + 1, 0:512])
                else:
                    nc.scalar.copy(sgq[qd][:, :],
                                   sgrid[32 * qd:32 * qd + 1, 0:512])
                for aa in range(4):
                    j = 4 * qd + aa
                    nc.tensor.matmul(
                        wtp[:, j:j + 1],
                        lhsT=sgq[qd][0:1, 128 * aa:128 * (aa + 1)],
                        rhs=one1[:, :],
                        is_transpose=True, start=(j == 0), stop=(j == NSL - 1),
                    )
            wcolT = const.tile([128, NSL], F16, tag="wcolT")
            nc.vector.tensor_tensor(
                out=wcolT[:, :], in0=wtp[:, 0:NSL], in1=efT[:, :], op=MULT,
            )

            # ---- pass 2: recip[n] = sum_j csT[:, j, n]^T wcolT[:, j] ----
            rr = wk_out.tile([128, NCH], F32, name="rr")
            p2_order = list(range(NCH // 2))
            if p2_desc:
                p2_order = p2_order[::-1]
            for m in p2_order:
                rp = pu.tile([128, KSH], F32, tag="u", name=f"rp{m}")
                for c in (0, 1):
                    t = 2 * m + c
                    for j in range(NSL):
                        nc.tensor.matmul(
                            rp[:, 512 * c:512 * c + 1],
                            lhsT=csT[:, j, 128 * t:128 * (t + 1)],
                            rhs=wcolT[:, j:j + 1],
                            start=(j == 0), stop=(j == NSL - 1),
                        )
                if m % 4 < p2_act:
                    nc.scalar.copy(rr[:, 2 * m:2 * m + 2], rp[:, 0:KSH:512])
                else:
                    nc.vector.tensor_copy(rr[:, 2 * m:2 * m + 2],
                                          rp[:, 0:KSH:512])
                if out_pieces == 2 and m == 11:
                    nc.sync.dma_start(out=recip_d[:, 0:24], in_=rr[:, 0:24])
            if out_pieces == 2:
                nc.sync.dma_start(out=recip_d[:, 24:], in_=rr[:, 24:])
            else:
                nc.sync.dma_start(out=recip_d, in_=rr[:, :])

    nc.compile()
    return nc


def _get_prog(reps: int = 1, stage: str = "full", **kw):
    key = (reps, stage, tuple(sorted(kw.items())))
    if key not in _PROG:
        _PROG[key] = _build_program(reps, stage, **kw)
    return _PROG[key]


def _make_in_maps(coords, q, cell_inv, kvec, expfac):
    in_maps = []
    for c in range(NCORES):
        b, ks = divmod(c, NCORES // B)
        sl = slice(KSH * ks, KSH * (ks + 1))
        # efT[p, j] = expfac_shard[(128j + p) mod 1024]
        ef = np.asarray(expfac[sl], dtype=np.float32).reshape(8, 128).T  # [128, 8]
        efT = np.concatenate([ef, ef], axis=1)                           # [128, 16]
        in_maps.append({
            "coordsT": np.ascontiguousarray(coords[b].T, dtype=np.float32),
            "qT": np.ascontiguousarray(q[b].reshape(NCH, 128).T, dtype=np.float32),
            "kmodT": np.ascontiguousarray(
                cell_inv.astype(np.float32).T @ kvec[sl].T.astype(np.float32)),
            "efT": np.ascontiguousarray(efT),
        })
    return in_maps


def _finalize(results, q, volume, bewald):
    recip = np.zeros((B, N), np.float32)
    for c in range(NCORES):
        b = c // (NCORES // B)
        recip[b] += results[c]["recip"].T.reshape(-1)
    scale1 = np.float32(BOHR / (math.pi * float(volume[0])))
    scale2 = np.float32(2.0 * float(bewald[0]) * BOHR / math.sqrt(math.pi))
    phi = (recip * scale1 - q.astype(np.float32) * scale2).astype(np.float32)
    e = (np.float32(0.5) * q.astype(np.float32) * phi).astype(np.float32)
    return e, phi


def kernel(coords, q, cell_inv, kvec, expfac, volume, bewald):
    from concourse.bass_utils import run_bass_kernel_spmd

    nc = _get_prog()
    in_maps = _make_in_maps(coords, q, cell_inv, kvec, expfac)
    res = run_bass_kernel_spmd(nc, in_maps, list(range(NCORES))).results
    return _finalize(res, q, volume, bewald)


# revision 13
# speedup vs baseline: 1.0055x; 1.0027x over previous
"""Ewald reciprocal-space kernel for Trainium2 (8 NeuronCores, SPMD) — v3.

Math (per batch b):
    s        = cell_inv @ x          (fractional coords)
    theta    = 2*pi * (kvec . s)     (B, N, NK) phases
    S_re/S_im= sum_n q_n {cos,sin}(theta)          (structure factor)
    recip_n  = sum_k expfac_k (S_re cos + S_im sin)
    phi      = recip * BOHR/(pi*V) - q * 2*bewald*BOHR/sqrt(pi)
    returns (0.5*q*phi, phi)

Sharding: 8 cores = 2 batches x 4 k-shards (1024 k-vectors each). Each core
computes its full-N, shard-K contribution to recip with no collectives; host
sums the 4 shard partials per batch and applies the final affine.

Device pipeline per core (N=4096 as 32 chunks of 128 partitions):
  u  = x . kmod (kmod = Cinv^T k)       fp32r matmul into PSUM
  rn = (u + M) - M                      magic round; DVE or ACT-assisted
  f  = rn - u  in [-1/2, 1/2]           DVE scalar_tensor_tensor, fp16 out
  g  = |f|     (uint16 AND 0x7FFF)      DVE 4x-mode fp16
  h  = g - 1/4                          GPSIMD (pool), fp16
  sin(theta) = Sin(-2pi f), cos(theta) = Sin(-2pi h)   one ACT Sin per 2 chunks
  S rows: 4 PE matmuls/chunk, out [1, 512] parked at quadrant partition 32q
     (4 concurrent PSUM groups share one 2KB region on disjoint partitions)
  cs chunks DMA-transposed (xbar) into csT[k-slice partitions, n free]
  S extraction: 4 single-partition row copies (DVE+ACT), then 16 tiny PE
     transposes [1,128]->[128,1] into one PSUM group; w = efT * S^T (DVE)
  recip: free-size-1 weight-stationary PE matmuls (lhsT = csT block,
     rhs = w column), accumulated over 16 k-slices; PSUM ring reuse with
     per-pair copy-out, recip emitted as [128, 32]
"""

import math
from contextlib import ExitStack

import numpy as np

BOHR = 1.8897261258369282

B, N, NK = 2, 4096, 4096
NCORES = 8
KSH = NK // 4          # k-vectors per core
NCH = N // 128         # 32 n-chunks
CW = 2 * KSH           # cs chunk width: [sin | cos]
NSL = CW // 128        # 16 k-slices per chunk
MAGIC = 12582912.0     # 1.5 * 2**23: fp32 round-to-nearest-integer
NEG2PI = -6.28318452835083  # two fp32 ulps below 2*pi

_PROG = {}


def _build_program(reps: int = 1, stage: str = 'full', n_assist: int = 14,
                   mm_bufs: int = 3, rn_bufs: int = 2, cs_bufs: int = 2,
                   extr_act: int = 4, p2_act: int = 1, assist_last: int = NCH,
                   assist0: bool = False,
                   split_sins: tuple = (NCH // 2 - 2, NCH // 2 - 1),
                   out_pieces: int = 1, p2_desc: bool = False,
                   assist_set: frozenset = None, pre_u: int = 3,
                   hoist: bool = False, extr_rev: bool = False,
                   last_half: int = 1):
    import concourse.bass as bass
    import concourse.bacc as bacc
    import concourse.tile as tile
    import concourse.mybir as mybir

    F32 = mybir.dt.float32
    F32R = mybir.dt.float32r
    F16 = mybir.dt.float16
    U16 = mybir.dt.uint16
    ADD = mybir.AluOpType.add
    SUB = mybir.AluOpType.subtract
    MULT = mybir.AluOpType.mult
    AND = mybir.AluOpType.bitwise_and
    SIN = mybir.ActivationFunctionType.Sin
    COPY = mybir.ActivationFunctionType.Copy

    nc = bacc.Bacc(trn_type="TRN2", target_bir_lowering=False, debug=False)

    coordsT_d = nc.dram_tensor("coordsT", [3, N], F32, kind="ExternalInput").ap()
    qT_d = nc.dram_tensor("qT", [128, NCH], F32, kind="ExternalInput").ap()
    kmodT_d = nc.dram_tensor("kmodT", [3, KSH], F32, kind="ExternalInput").ap()
    efT_d = nc.dram_tensor("efT", [128, NSL], F32, kind="ExternalInput").ap()
    recip_d = nc.dram_tensor("recip", [128, NCH], F32, kind="ExternalOutput").ap()

    # chunks whose rounding runs on the scalar engine (DVE/ACT balance);
    # assist_last bounds the last assisted chunk so the ACT tail is Sin-only
    ASSIST = {int((i + 0.5) * assist_last / n_assist) for i in range(n_assist)}
    if assist0:
        ASSIST.add(0)
    if assist_set is not None:
        ASSIST = set(assist_set)

    with tile.TileContext(nc) as tc, ExitStack() as ctx:
        const = ctx.enter_context(tc.tile_pool(name="const", bufs=1))
        pu = ctx.enter_context(tc.tile_pool(name="pu", bufs=3, space="PSUM"))
        pacc = ctx.enter_context(tc.tile_pool(name="pacc", bufs=1, space="PSUM"))
        wk_rn = ctx.enter_context(tc.tile_pool(name="wk_rn", bufs=rn_bufs))
        wk_mm = ctx.enter_context(tc.tile_pool(name="wk_mm", bufs=mm_bufs))
        wk_cs = ctx.enter_context(tc.tile_pool(name="wk_cs", bufs=cs_bufs))
        wk_out = ctx.enter_context(tc.tile_pool(name="wk_out", bufs=1))

        # ---- load inputs (first chunk's dependencies in tiny pieces first) ----
        cts = const.tile([3, N], F32R)
        kmod = const.tile([3, KSH], F32R, name="kmod")
        nc.sync.dma_start(out=kmod[:, 0:512], in_=kmodT_d[:, 0:512].bitcast(F32R))
        nc.scalar.dma_start(out=cts[:, 0:384], in_=coordsT_d[:, 0:384].bitcast(F32R))
        nc.sync.dma_start(out=kmod[:, 512:1024],
                          in_=kmodT_d[:, 512:1024].bitcast(F32R))
        nc.sync.dma_start(out=cts[:, 384:1024],
                          in_=coordsT_d[:, 384:1024].bitcast(F32R))
        for hh in range(1024, N, 1024):
            nc.sync.dma_start(
                out=cts[:, hh:hh + 1024],
                in_=coordsT_d[:, hh:hh + 1024].bitcast(F32R),
            )
        qt = const.tile([128, NCH], F32)
        nc.sync.dma_start(out=qt[:, :], in_=qT_d)
        efT = const.tile([128, NSL], F32)
        nc.sync.dma_start(out=efT[:, :], in_=efT_d)
        qt16 = const.tile([128, NCH], F16)
        one1 = const.tile([1, 1], F32)

        # persistent: the transposed trig store and the S quadrant rows
        csT = const.tile([128, NSL, N], F16)   # [k-in-slice][slice j][n]
        sgrid = pacc.tile([128, 512], F32, tag="sg")

        vs = {}

        def emit_u(t):
            u_ps = pu.tile([128, KSH], F32, tag="u", name=f"u{t}")
            for hh in range(0, KSH, 512):
                nc.tensor.matmul(
                    u_ps[:, hh:hh + 512],
                    lhsT=cts[:, 128 * t:128 * (t + 1)],
                    rhs=kmod[:, hh:hh + 512],
                    start=True, stop=True,
                )
            if hoist and t in ASSIST and stage != 'rr0':
                # emit the ACT rounding assist at the producer so it can fill
                # ACT queue gaps ahead of the pair's Sin
                rnh = wk_rn.tile([128, KSH], F32, tag="rn", name=f"rnh{t}")
                nc.scalar.activation(rnh[:, :], u_ps[:, :], COPY,
                                     bias=MAGIC, scale=1.0)
                vs[t] = rnh
            return u_ps

        us = {t: emit_u(t) for t in range(pre_u)}
        for s in range(NCH // 2):
            mmp = wk_mm.tile([128, 2 * CW], F16, tag="mm", name=f"mm{s}")
            for c in (0, 1):
                t = 2 * s + c
                u_ps = us.pop(t)
                fsl = mmp[:, CW * c:CW * c + KSH]            # f (sin half)
                hsl = mmp[:, CW * c + KSH:CW * (c + 1)]      # h (cos half)
                if t in vs:
                    rn = vs.pop(t)
                    s0 = MAGIC
                elif t in ASSIST and stage != 'rr0':
                    # ACT: v = u + M (rounds); DVE stt: (v - M) - u
                    rn = wk_rn.tile([128, KSH], F32, tag="rn", name=f"rn{t}")
                    nc.scalar.activation(rn[:, :], u_ps[:, :], COPY,
                                         bias=MAGIC, scale=1.0)
                    s0 = MAGIC
                else:
                    rn = None
                    rn = wk_rn.tile([128, KSH], F32, tag="rn", name=f"rn{t}")
                    nc.vector.tensor_scalar(
                        out=rn[:, :], in0=u_ps[:, :],
                        scalar1=MAGIC, scalar2=MAGIC, op0=ADD, op1=SUB,
                    )
                    s0 = 0.0
                nc.vector.scalar_tensor_tensor(
                    out=fsl, in0=rn[:, :], scalar=s0, in1=u_ps[:, :],
                    op0=ADD if s0 == 0.0 else SUB, op1=SUB,
                )
                # g = |f| via uint16 AND; h = g - 1/4 on the pool engine
                nc.vector.tensor_scalar(
                    out=hsl.bitcast(U16), in0=fsl.bitcast(U16),
                    scalar1=0x7FFF, scalar2=None, op0=AND,
                )
                nc.gpsimd.tensor_scalar(
                    out=hsl, in0=hsl, scalar1=0.25, scalar2=None, op0=SUB,
                )
                tn = t + pre_u
                if tn < NCH:
                    us[tn] = emit_u(tn)
            if s == 0:
                nc.vector.tensor_copy(qt16[:, :], qt[:, :])
                nc.vector.memset(one1[:, :], 1.0)
            if stage == 'rr':
                continue
            # sin(theta) = Sin(-2pi f); cos(theta) = Sin(-2pi h)
            csp = wk_cs.tile([128, 2 * CW], F16, tag="cs", name=f"cs{s}")
            if s in split_sins:
                nc.scalar.activation(csp[:, 0:CW], mmp[:, 0:CW], SIN,
                                     bias=0.0, scale=NEG2PI)
            else:
                nc.scalar.activation(csp[:, :], mmp[:, :], SIN,
                                     bias=0.0, scale=NEG2PI)
            for c in (0, 1):
                t = 2 * s + c
                if s in split_sins and c == 1:
                    if last_half >= 2 or (last_half and s == NCH // 2 - 1):
                        # final chunk: sin-half Sin fires before the pool-sub
                        # finishes the cos half, releasing quadrant-0/1 S stops
                        nc.scalar.activation(csp[:, CW:CW + KSH],
                                             mmp[:, CW:CW + KSH], SIN,
                                             bias=0.0, scale=NEG2PI)
                        nc.scalar.activation(csp[:, CW + KSH:],
                                             mmp[:, CW + KSH:], SIN,
                                             bias=0.0, scale=NEG2PI)
                    else:
                        nc.scalar.activation(csp[:, CW:], mmp[:, CW:], SIN,
                                             bias=0.0, scale=NEG2PI)
                for qd in range(4):
                    nc.tensor.matmul(
                        sgrid[32 * qd:32 * qd + 1, 0:512],
                        lhsT=qt16[:, t:t + 1],
                        rhs=csp[:, CW * c + 512 * qd:CW * c + 512 * (qd + 1)],
                        start=(t == 0), stop=(t == NCH - 1),
                        tile_position=(0, 32 * qd),
                    )
                if stage != 'act':
                    # csT[p, j, 128t + n] = csp[n, CW*c + 128j + p]
                    nc.sync.dma_start_transpose(
                        out=csT[:, :, 128 * t:128 * (t + 1)],
                        in_=csp[:, CW * c:CW * (c + 1)],
                    )

        if stage != 'full':
            zz = wk_out.tile([128, NCH], F32, name="zz")
            nc.vector.memset(zz[:, :], 0.0)
            nc.sync.dma_start(out=recip_d, in_=zz[:, :])
        else:
            # ---- S extraction: 4 quadrant rows -> SBUF (DVE + ACT) ----
            sgq0 = const.tile([1, 512], F32, name="sgq0")
            sgq1 = const.tile([1, 512], F32, name="sgq1")
            sgq2 = const.tile([1, 512], F32, name="sgq2")
            sgq3 = const.tile([1, 512], F32, name="sgq3")
            sgq = [sgq0, sgq1, sgq2, sgq3]
            wtp = pacc.tile([128, 512], F32, tag="wtp")
            for qd in range(4):
                on_act = (qd < extr_act) != extr_rev
                if not on_act:
                    nc.vector.tensor_copy(sgq[qd][:, :],
                                          sgrid[32 * qd:32 * qd + 1, 0:512])
                else:
                    nc.scalar.copy(sgq[qd][:, :],
                                   sgrid[32 * qd:32 * qd + 1, 0:512])
                for aa in range(4):
                    j = 4 * qd + aa
                    nc.tensor.matmul(
                        wtp[:, j:j + 1],
                        lhsT=sgq[qd][0:1, 128 * aa:128 * (aa + 1)],
                        rhs=one1[:, :],
                        is_transpose=True, start=(j == 0), stop=(j == NSL - 1),
                    )
            wcolT = const.tile([128, NSL], F16, tag="wcolT")
            nc.vector.tensor_tensor(
                out=wcolT[:, :], in0=wtp[:, 0:NSL], in1=efT[:, :], op=MULT,
            )

            # ---- pass 2: recip[n] = sum_j csT[:, j, n]^T wcolT[:, j] ----
            rr = wk_out.tile([128, NCH], F32, name="rr")
            p2_order = list(range(NCH // 2))
            if p2_desc:
                p2_order = p2_order[::-1]
            for m in p2_order:
                rp = pu.tile([128, KSH], F32, tag="u", name=f"rp{m}")
                for c in (0, 1):
                    t = 2 * m + c
                    for j in range(NSL):
                        nc.tensor.matmul(
                            rp[:, 512 * c:512 * c + 1],
                            lhsT=csT[:, j, 128 * t:128 * (t + 1)],
                            rhs=wcolT[:, j:j + 1],
                            start=(j == 0), stop=(j == NSL - 1),
                        )
                if m % 4 < p2_act:
                    nc.scalar.copy(rr[:, 2 * m:2 * m + 2], rp[:, 0:KSH:512])
                else:
                    nc.vector.tensor_copy(rr[:, 2 * m:2 * m + 2],
                                          rp[:, 0:KSH:512])
                if out_pieces == 2 and m == 11:
                    nc.sync.dma_start(out=recip_d[:, 0:24], in_=rr[:, 0:24])
            if out_pieces == 2:
                nc.sync.dma_start(out=recip_d[:, 24:], in_=rr[:, 24:])
            else:
                nc.sync.dma_start(out=recip_d, in_=rr[:, :])

    nc.compile()
    return nc


def _get_prog(reps: int = 1, stage: str = "full", **kw):
    key = (reps, stage, tuple(sorted(kw.items())))
    if key not in _PROG:
        _PROG[key] = _build_program(reps, stage, **kw)
    return _PROG[key]


def _make_in_maps(coords, q, cell_inv, kvec, expfac):
    in_maps = []
    for c in range(NCORES):
        b, ks = divmod(c, NCORES // B)
        sl = slice(KSH * ks, KSH * (ks + 1))
        # efT[p, j] = expfac_shard[(128j + p) mod 1024]
        ef = np.asarray(expfac[sl], dtype=np.float32).reshape(8, 128).T  # [128, 8]
        efT = np.concatenate([ef, ef], axis=1)                           # [128, 16]
        in_maps.append({
            "coordsT": np.ascontiguousarray(coords[b].T, dtype=np.float32),
            "qT": np.ascontiguousarray(q[b].reshape(NCH, 128).T, dtype=np.float32),
            "kmodT": np.ascontiguousarray(
                cell_inv.astype(np.float32).T @ kvec[sl].T.astype(np.float32)),
            "efT": np.ascontiguousarray(efT),
        })
    return in_maps


def _finalize(results, q, volume, bewald):
    recip = np.zeros((B, N), np.float32)
    for c in range(NCORES):
        b = c // (NCORES // B)
        recip[b] += results[c]["recip"].T.reshape(-1)
    scale1 = np.float32(BOHR / (math.pi * float(volume[0])))
    scale2 = np.float32(2.0 * float(bewald[0]) * BOHR / math.sqrt(math.pi))
    phi = (recip * scale1 - q.astype(np.float32) * scale2).astype(np.float32)
    e = (np.float32(0.5) * q.astype(np.float32) * phi).astype(np.float32)
    return e, phi


def kernel(coords, q, cell_inv, kvec, expfac, volume, bewald):
    from concourse.bass_utils import run_bass_kernel_spmd

    nc = _get_prog()
    in_maps = _make_in_maps(coords, q, cell_inv, kvec, expfac)
    res = run_bass_kernel_spmd(nc, in_maps, list(range(NCORES))).results
    return _finalize(res, q, volume, bewald)
